# revision 27
# baseline (speedup 1.0000x reference)
"""Trainium2 Bass kernel for ParallelLMHeadWithLoRA.

logits = hidden @ W^T + (hidden @ A^T) @ B^T
  hidden [2048, 4096] f32, W [32000, 4096] f32, A [16, 4096], B [32000, 16]

Strategy (8 NeuronCores, tensor-parallel over vocab):
  - Host folds the LoRA into the weights: W' = W + B @ A (exact algebra,
    one 4.2-GFLOP sgemm on host). The device kernel is then a single
    dense matmul h @ W'^T.
  - Each core owns a 4000-wide vocab slice of W', split into 32 blocks
    of 125 columns (125, not 128: 128-column fp16 weight loads trigger
    the 4-XBUS fast-weight-load path, measured SLOWER here).
  - Mixed precision over the contraction dim (D = 32 chunks of 128):
      * KF chunks in fp16 at 1 row/cycle (512-cycle matmuls)
      * K8 = 32-KF chunks in fp8-e4m3 using DoubleRow perf mode, which
        contracts 2 chunks per matmul at 2 fp8 MACs/cell/cycle.
    A DoubleRow matmul at 512 output columns occupies the same
    512-cycle slot as an fp16 matmul (the moving stream is XBUS-limited
    at 16 bits/partition/cycle), so it contracts 2 chunks per slot.
    fp8 operand scaling: h8 = e4m3(h*2^5), W8 = e4m3(W'*2^10), so the
    fp8 partial sums carry scale 2^15. The fp16 weights are pre-scaled
    by 2^15 (exact, power of two) so ALL partial sums in PSUM share
    scale 2^15; eviction multiplies by 2^-15 (DVE tensor_scalar_mul,
    same cost as the plain copy) and writes fp16 (upcast to f32 on
    host). Exact end-to-end error vs the fp64 reference, measured on
    the real inputs: 1.59e-2 (K8=8) against the 2e-2 gate; K8=10 hits
    1.999e-2 absmax/scale -- too close. Pure fp16 is 2.5e-4.
  - hidden^T stays SBUF-resident; W' streams through once. The fp8 DR
    groups run FIRST in every vb: their operands are small (2.1 MB
    total) and land early, so the DR phase doubles as PE warm-up (HAM
    un-throttle) while the 12.3 MB fp16 hidden stream spins up. vb 0
    and 1 are interleaved at dc granularity (8 PSUM banks); the last vb
    runs tb-major so its output drains during its own matmuls.
  - Per-matmul steady state measured 216 ns (512 cycles @ 2.4 GHz +
    ~3 ns issue); LDWEIGHTS fully hides behind the matmul stream.
    3584 matmuls/core -> ~775 us floor, ~810 us measured end-to-end.
    NOTE: the chip sometimes runs in a 2.0 GHz power state; the same
    binary then measures ~20% slower. Nothing kernel-side controls it.
"""

import numpy as np
import ml_dtypes

import concourse.mybir as mybir
import concourse.tile as tile
from concourse import bacc
from concourse.bass_utils import run_bass_kernel_spmd

P = 128
N_TOK = 2048
D = 4096
V = 32000
R = 16
NCORES = 8

VC = V // NCORES          # 4000 vocab per core
VBS = 125                 # vocab block (psum partition dim)
VB = VC // VBS            # 32 vocab blocks
DC = D // P               # 32 contraction chunks
TBS = 512                 # moving free dim per matmul (PSUM bank cap)
TB = N_TOK // TBS         # 4 token blocks

K8 = 8                    # fp8 contraction chunks (even; 0 disables fp8)
KF = DC - K8              # fp16 contraction chunks
G = K8 // 2               # fp8 DoubleRow groups (2 chunks each)

PSUM_SCALE = 2.0 ** 15    # fp16 W pre-scale == fp8 h_scale * w_scale
H8_SCALE = 2.0 ** 5
W8_SCALE = 2.0 ** 10

F32 = mybir.dt.float32
F16 = mybir.dt.float16
F8 = mybir.dt.float8e4
E4M3 = ml_dtypes.float8_e4m3fn

DR = mybir.MatmulPerfMode.DoubleRow


def build_nc(wt_bufs=3, out_bufs=4):
    nc = bacc.Bacc(None, target_bir_lowering=False, debug=False)

    wtb = nc.dram_tensor("wtb", [VB, P, KF, VBS], F16, kind="ExternalInput")
    htt = nc.dram_tensor("htt", [KF * P, N_TOK], F16, kind="ExternalInput")
    if G:
        wtb8 = nc.dram_tensor("wtb8", [VB, P, 2 * G, P], F8, kind="ExternalInput")
        htt8 = nc.dram_tensor("htt8", [G, P, 2, N_TOK], F8, kind="ExternalInput")
    outt = nc.dram_tensor("outt", [VC, N_TOK], F16, kind="ExternalOutput")

    with tile.TileContext(nc) as tc:
        WHEAD = 4
        HSPLIT = 2
        with (
            tc.tile_pool(name="htp", bufs=KF - HSPLIT) as htp,
            tc.tile_pool(name="hthp", bufs=2 * HSPLIT) as hthp,
            tc.tile_pool(name="ht8p", bufs=max(G, 1)) as ht8p,
            tc.tile_pool(name="wtp", bufs=wt_bufs) as wtp,
            tc.tile_pool(name="wthp", bufs=2) as wthp,
            tc.tile_pool(name="wtbp", bufs=2) as wtbp,
            tc.tile_pool(name="wt8p", bufs=wt_bufs) as wt8p,
            tc.tile_pool(name="outp", bufs=out_bufs) as outp,
            tc.tile_pool(name="psp", bufs=8, space="PSUM") as psp,
        ):
            # weight blocks for the interleaved pair go out first on the
            # sync queue; ht streams on the gpsimd queue in parallel;
            # the small fp8 ht goes on the vector queue.
            wt_tiles, wt8_tiles = {}, {}

            def fetch_wt(vb):
                w8_t = None
                if G:
                    w8_t = wt8p.tile([P, 2 * G, P], F8, name="w8_t", tag="w8")
                    nc.sync.dma_start(w8_t[:], wtb8[vb, :, :, :])
                wt_t = wtp.tile([P, KF, VBS], F16, name="wt_t", tag="wt")
                nc.sync.dma_start(wt_t[:], wtb[vb, :, :, :])
                return wt_t, w8_t

            # Startup: dependencies are kept fine-grained so the PE can
            # start as soon as the first small pieces land. vb0/vb1 get
            # their weights as a 4-chunk head tile + body tile; the first
            # HSPLIT ht chunks come as half-token tiles spread over the
            # scalar/sync/gpsimd queues while they spin up.
            ht_halves = {}
            for dc in range(HSPLIT):
                ht_halves[dc] = [
                    hthp.tile([P, N_TOK // 2], F16, name=f"hth_{dc}_{h}", tag="hth")
                    for h in range(2)
                ]
            ht_tiles = {
                dc: htp.tile([P, N_TOK], F16, name=f"ht_{dc}", tag="ht")
                for dc in range(HSPLIT, KF)
            }

            def dma_ht_half(eng, dc, h):
                eng.dma_start(
                    ht_halves[dc][h][:],
                    htt[dc * P:(dc + 1) * P,
                        h * (N_TOK // 2):(h + 1) * (N_TOK // 2)],
                )

            wt_head, wt_body = {}, {}

            def fetch_wt01(vb):
                wh = wthp.tile([P, WHEAD, VBS], F16, name="wt_h", tag="wth")
                nc.sync.dma_start(wh[:], wtb[vb, :, 0:WHEAD, :])
                wb = wtbp.tile([P, KF - WHEAD, VBS], F16, name="wt_b", tag="wtb")
                nc.sync.dma_start(wb[:], wtb[vb, :, WHEAD:KF, :])
                wt_head[vb], wt_body[vb] = wh, wb

            # The DR (fp8) groups run first in every vb, so their small
            # operands lead each queue: wt8 pair + 2 ht8 tiles on sync,
            # one ht8 on gpsimd and one on scalar. The big fp16 stream
            # follows and has the DR phase as runway.
            ht8_tiles = [
                ht8p.tile([P, 2, N_TOK], F8, name=f"ht8_{g}", tag="ht8")
                for g in range(G)
            ]
            w8_0 = wt8p.tile([P, 2 * G, P], F8, name="w8_t", tag="w8")
            nc.sync.dma_start(w8_0[:], wtb8[0, :, :, :])
            nc.gpsimd.dma_start(ht8_tiles[0][:], htt8[0, :, :, :])
            nc.scalar.dma_start(ht8_tiles[1][:], htt8[1, :, :, :])
            w8_1 = wt8p.tile([P, 2 * G, P], F8, name="w8_t", tag="w8")
            nc.sync.dma_start(w8_1[:], wtb8[1, :, :, :])
            for g in range(2, G):
                nc.sync.dma_start(ht8_tiles[g][:], htt8[g, :, :, :])
            wt8_tiles[0], wt8_tiles[1] = w8_0, w8_1

            dma_ht_half(nc.gpsimd, 0, 0)
            dma_ht_half(nc.scalar, 1, 0)
            fetch_wt01(0)
            dma_ht_half(nc.gpsimd, 0, 1)
            dma_ht_half(nc.scalar, 1, 1)
            fetch_wt01(1)

            for dc in range(HSPLIT, KF):
                eng = nc.gpsimd if dc % 2 == 0 else nc.scalar
                eng.dma_start(ht_tiles[dc][:], htt[dc * P:(dc + 1) * P, :])

            def ht_slice(dc, tb):
                if dc < HSPLIT:
                    half = ht_halves[dc][tb // 2]
                    r = tb % 2
                    return half[:, r * TBS:(r + 1) * TBS]
                return ht_tiles[dc][:, tb * TBS:(tb + 1) * TBS]

            def wt_slice(vb, wt_t, dc):
                if vb in wt_head:
                    if dc < WHEAD:
                        return wt_head[vb][:, dc, :]
                    return wt_body[vb][:, dc - WHEAD, :]
                return wt_t[:, dc, :]

            def mm(vb, pss, wt_t, w8_t, dc, tb):
                """dc in [0, KF) -> fp16 chunk; dc in [KF, KF+G) -> fp8 pair."""
                ts0 = tb * TBS
                if dc < KF:
                    nc.tensor.matmul(
                        pss[tb][:],
                        wt_slice(vb, wt_t, dc),
                        ht_slice(dc, tb),
                        start=(G == 0 and dc == 0),
                        stop=(dc == KF - 1),
                    )
                else:
                    g = dc - KF
                    nc.tensor.matmul(
                        pss[tb][:],
                        w8_t[:, 2 * g:2 * g + 2, 0:VBS],
                        ht8_tiles[g][:, :, ts0:ts0 + TBS],
                        start=(g == 0),
                        stop=False,
                        perf_mode=DR,
                    )

            def evict_tb(vb, pss, tb):
                ts0 = tb * TBS
                ot = outp.tile([VBS, TBS], F16, name="ot", tag="ot")
                nc.vector.tensor_scalar_mul(ot[:], pss[tb][:], 1.0 / PSUM_SCALE)
                # alternate output DMAs over two queues for 2x drain rate
                eng = nc.scalar if tb % 2 == 0 else nc.gpsimd
                eng.dma_start(
                    outt[vb * VBS:(vb + 1) * VBS, ts0:ts0 + TBS], ot[:]
                )

            SEQ = list(range(KF, KF + G)) + list(range(KF))

            # vb 0+1 interleaved at dc granularity: PE does 8 matmuls per
            # arriving ht chunk, tracking the DMA stream without stalling.
            # vb1 is staggered 2 chunks behind vb0 so the early matmuls
            # only depend on wt0 while wt1 is still in flight.
            pss01 = [
                [psp.tile([VBS, TBS], F32, name=f"ps{v}_{tb}", tag="ps")
                 for tb in range(TB)]
                for v in range(2)
            ]
            for d in SEQ:
                for v in range(2):
                    for tb in range(TB):
                        mm(v, pss01[v], None, wt8_tiles[v], d, tb)
            for v in range(2):
                for tb in range(TB):
                    evict_tb(v, pss01[v], tb)

            for vb in range(2, VB):
                wt_t, w8_t = fetch_wt(vb)
                pss = [
                    psp.tile([VBS, TBS], F32, name=f"ps{tb}", tag="ps")
                    for tb in range(TB)
                ]
                if vb < VB - 1:
                    for dc in SEQ:
                        for tb in range(TB):
                            mm(vb, pss, wt_t, w8_t, dc, tb)
                    for tb in range(TB):
                        evict_tb(vb, pss, tb)
                else:
                    # last vb tb-major: each token block finishes early so
                    # its eviction + output DMA overlap the remaining MMs;
                    # the final block drains in quarters to shorten the tail
                    for tb in range(TB):
                        for dc in SEQ:
                            mm(vb, pss, wt_t, w8_t, dc, tb)
                        if tb < TB - 1:
                            evict_tb(vb, pss, tb)
                        else:
                            ts0 = tb * TBS
                            for q in range(4):
                                qs = q * (TBS // 4)
                                ot = outp.tile([VBS, TBS // 4], F16,
                                               name="otq", tag="ot")
                                nc.vector.tensor_scalar_mul(
                                    ot[:], pss[tb][:, qs:qs + TBS // 4],
                                    1.0 / PSUM_SCALE,
                                )
                                eng = nc.scalar if q % 2 == 0 else nc.gpsimd
                                eng.dma_start(
                                    outt[vb * VBS:(vb + 1) * VBS,
                                         ts0 + qs:ts0 + qs + TBS // 4],
                                    ot[:],
                                )
    nc.compile()
    return nc


def _q8(x, scale):
    return np.clip(x * scale, -240.0, 240.0).astype(E4M3)


def _prep_inputs(hidden_states, weight, lora_A, lora_B):
    wf = np.asarray(weight, dtype=np.float32) + (
        np.asarray(lora_B, dtype=np.float32) @ np.asarray(lora_A, dtype=np.float32)
    )
    h32 = np.asarray(hidden_states, dtype=np.float32)

    # fp16 part: chunks [0, KF), weights pre-scaled by PSUM_SCALE
    ws = (wf[:, :KF * P] * np.float32(PSUM_SCALE)).astype(np.float16)
    # [core, vb, j, dc, p] -> [core, vb, p, dc, j]
    wtb_all = np.ascontiguousarray(
        ws.reshape(NCORES, VB, VBS, KF, P).transpose(0, 1, 4, 3, 2)
    )
    htt = np.ascontiguousarray(h32[:, :KF * P].astype(np.float16).T)

    maps = [{"wtb": wtb_all[c], "htt": htt} for c in range(NCORES)]

    if G:
        # fp8 part: chunks [KF, DC) as G DoubleRow pairs
        w8 = _q8(wf[:, KF * P:], W8_SCALE)          # [V, K8*P]
        # [c, vb, j, g, ko, p] -> [c, vb, p, (g,ko), j] padded j: 125->128
        w8b = w8.reshape(NCORES, VB, VBS, G, 2, P).transpose(0, 1, 5, 3, 4, 2)
        w8b = np.ascontiguousarray(
            np.pad(w8b, [(0, 0)] * 5 + [(0, P - VBS)])
            .reshape(NCORES, VB, P, 2 * G, P)
        )
        h8 = _q8(h32[:, KF * P:], H8_SCALE)         # [N, K8*P]
        # -> [g, ko, p, t] -> [g, p, ko, t]
        htt8 = np.ascontiguousarray(
            h8.T.reshape(G, 2, P, N_TOK).transpose(0, 2, 1, 3)
        )
        for c in range(NCORES):
            maps[c]["wtb8"] = w8b[c]
            maps[c]["htt8"] = htt8

    return maps


def run(hidden_states, weight, lora_A, lora_B, trace=False, **run_kwargs):
    in_maps = _prep_inputs(hidden_states, weight, lora_A, lora_B)
    nc = build_nc()
    res = run_bass_kernel_spmd(
        nc, in_maps, core_ids=list(range(NCORES)), trace=trace, **run_kwargs
    )
    out = np.empty((N_TOK, V), dtype=np.float32)
    for c in range(NCORES):
        out[:, c * VC:(c + 1) * VC] = res.results[c]["outt"].T.astype(np.float32)
    return out, res


def kernel(hidden_states, weight, lora_A, lora_B):
    out, _ = run(hidden_states, weight, lora_A, lora_B, trace=False)
    return out


# revision 28
# speedup vs baseline: 1.0069x; 1.0069x over previous
"""Trainium2 Bass kernel for ParallelLMHeadWithLoRA.

logits = hidden @ W^T + (hidden @ A^T) @ B^T
  hidden [2048, 4096] f32, W [32000, 4096] f32, A [16, 4096], B [32000, 16]

Strategy (8 NeuronCores, tensor-parallel over vocab):
  - Host folds the LoRA into the weights: W' = W + B @ A (exact algebra,
    one 4.2-GFLOP sgemm on host). The device kernel is then a single
    dense matmul h @ W'^T.
  - Each core owns a 4000-wide vocab slice of W', split into 32 blocks
    of 125 columns (125, not 128: 128-column fp16 weight loads trigger
    the 4-XBUS fast-weight-load path, measured SLOWER here).
  - Mixed precision over the contraction dim (D = 32 chunks of 128):
      * KF chunks in fp16 at 1 row/cycle (512-cycle matmuls)
      * K8 = 32-KF chunks in fp8-e4m3 using DoubleRow perf mode, which
        contracts 2 chunks per matmul at 2 fp8 MACs/cell/cycle.
    A DoubleRow matmul at 512 output columns occupies the same
    512-cycle slot as an fp16 matmul (the moving stream is XBUS-limited
    at 16 bits/partition/cycle), so it contracts 2 chunks per slot.
    fp8 operand scaling: h8 = e4m3(h*2^5), W8 = e4m3(W'*2^10), so the
    fp8 partial sums carry scale 2^15. The fp16 weights are pre-scaled
    by 2^15 (exact, power of two) so ALL partial sums in PSUM share
    scale 2^15; eviction multiplies by 2^-15 (DVE tensor_scalar_mul,
    same cost as the plain copy) and writes fp16 (upcast to f32 on
    host). Exact end-to-end error vs the fp64 reference, measured on
    the real inputs: 1.59e-2 (K8=8) against the 2e-2 gate; K8=10 hits
    1.999e-2 absmax/scale -- too close. Pure fp16 is 2.5e-4.
  - hidden^T stays SBUF-resident; W' streams through once. The fp8 DR
    groups run FIRST in every vb: their operands are small (2.1 MB
    total) and land early, so the DR phase doubles as PE warm-up (HAM
    un-throttle) while the 12.3 MB fp16 hidden stream spins up. vb 0
    and 1 are interleaved at dc granularity (8 PSUM banks); the last vb
    runs tb-major so its output drains during its own matmuls.
  - Per-matmul steady state measured 216 ns (512 cycles @ 2.4 GHz +
    ~3 ns issue); LDWEIGHTS fully hides behind the matmul stream.
    3584 matmuls/core -> ~775 us floor, ~810 us measured end-to-end.
    NOTE: the chip sometimes runs in a 2.0 GHz power state; the same
    binary then measures ~20% slower. Nothing kernel-side controls it.
"""

import numpy as np
import ml_dtypes

import concourse.mybir as mybir
import concourse.tile as tile
from concourse import bacc
from concourse.bass_utils import run_bass_kernel_spmd

P = 128
N_TOK = 2048
D = 4096
V = 32000
R = 16
NCORES = 8

VC = V // NCORES          # 4000 vocab per core
VBS = 125                 # vocab block (psum partition dim)
VB = VC // VBS            # 32 vocab blocks
DC = D // P               # 32 contraction chunks
TBS = 512                 # moving free dim per matmul (PSUM bank cap)
TB = N_TOK // TBS         # 4 token blocks

K8 = 8                    # fp8 contraction chunks (even; 0 disables fp8)
KF = DC - K8              # fp16 contraction chunks
G = K8 // 2               # fp8 DoubleRow groups (2 chunks each)

PSUM_SCALE = 2.0 ** 15    # fp16 W pre-scale == fp8 h_scale * w_scale
H8_SCALE = 2.0 ** 5
W8_SCALE = 2.0 ** 10

F32 = mybir.dt.float32
F16 = mybir.dt.float16
F8 = mybir.dt.float8e4
E4M3 = ml_dtypes.float8_e4m3fn

DR = mybir.MatmulPerfMode.DoubleRow


def build_nc(wt_bufs=3, out_bufs=4):
    nc = bacc.Bacc(None, target_bir_lowering=False, debug=False)

    wtb = nc.dram_tensor("wtb", [VB, P, KF, VBS], F16, kind="ExternalInput")
    htt = nc.dram_tensor("htt", [KF * P, N_TOK], F16, kind="ExternalInput")
    if G:
        wtb8 = nc.dram_tensor("wtb8", [VB, P, 2 * G, P], F8, kind="ExternalInput")
        htt8 = nc.dram_tensor("htt8", [G, P, 2, N_TOK], F8, kind="ExternalInput")
    outt = nc.dram_tensor("outt", [VC, N_TOK], F16, kind="ExternalOutput")

    with tile.TileContext(nc) as tc:
        WHEAD = 4
        HSPLIT = 2
        with (
            tc.tile_pool(name="htp", bufs=KF - HSPLIT) as htp,
            tc.tile_pool(name="hthp", bufs=2 * HSPLIT) as hthp,
            tc.tile_pool(name="ht8p", bufs=max(G, 1)) as ht8p,
            tc.tile_pool(name="wtp", bufs=wt_bufs) as wtp,
            tc.tile_pool(name="wthp", bufs=2) as wthp,
            tc.tile_pool(name="wtbp", bufs=2) as wtbp,
            tc.tile_pool(name="wt8p", bufs=wt_bufs) as wt8p,
            tc.tile_pool(name="outp", bufs=out_bufs) as outp,
            tc.tile_pool(name="psp", bufs=8, space="PSUM") as psp,
        ):
            # weight blocks for the interleaved pair go out first on the
            # sync queue; ht streams on the gpsimd queue in parallel;
            # the small fp8 ht goes on the vector queue.
            wt_tiles, wt8_tiles = {}, {}

            def fetch_wt(vb):
                w8_t = None
                if G:
                    w8_t = wt8p.tile([P, 2 * G, P], F8, name="w8_t", tag="w8")
                    nc.sync.dma_start(w8_t[:], wtb8[vb, :, :, :])
                wt_t = wtp.tile([P, KF, VBS], F16, name="wt_t", tag="wt")
                nc.sync.dma_start(wt_t[:], wtb[vb, :, :, :])
                return wt_t, w8_t

            # Startup: dependencies are kept fine-grained so the PE can
            # start as soon as the first small pieces land. vb0/vb1 get
            # their weights as a 4-chunk head tile + body tile; the first
            # HSPLIT ht chunks come as half-token tiles spread over the
            # scalar/sync/gpsimd queues while they spin up.
            ht_halves = {}
            for dc in range(HSPLIT):
                ht_halves[dc] = [
                    hthp.tile([P, N_TOK // 2], F16, name=f"hth_{dc}_{h}", tag="hth")
                    for h in range(2)
                ]
            ht_tiles = {
                dc: htp.tile([P, N_TOK], F16, name=f"ht_{dc}", tag="ht")
                for dc in range(HSPLIT, KF)
            }

            def dma_ht_half(eng, dc, h):
                eng.dma_start(
                    ht_halves[dc][h][:],
                    htt[dc * P:(dc + 1) * P,
                        h * (N_TOK // 2):(h + 1) * (N_TOK // 2)],
                )

            wt_head, wt_body = {}, {}

            # The DR (fp8) groups run first in every vb, so their small
            # operands lead each queue: wt8 pair + 2 ht8 tiles on sync,
            # one ht8 on gpsimd and one on scalar. The big fp16 stream
            # follows and has the DR phase as runway.
            ht8_tiles = [
                ht8p.tile([P, 2, N_TOK], F8, name=f"ht8_{g}", tag="ht8")
                for g in range(G)
            ]
            w8_0 = wt8p.tile([P, 2 * G, P], F8, name="w8_t", tag="w8")
            nc.sync.dma_start(w8_0[:], wtb8[0, :, :, :])
            nc.gpsimd.dma_start(ht8_tiles[0][:], htt8[0, :, :, :])
            nc.scalar.dma_start(ht8_tiles[1][:], htt8[1, :, :, :])
            w8_1 = wt8p.tile([P, 2 * G, P], F8, name="w8_t", tag="w8")
            nc.sync.dma_start(w8_1[:], wtb8[1, :, :, :])
            wt8_tiles[0], wt8_tiles[1] = w8_0, w8_1
            # remaining ht8 ride the side queues; the sync queue carries
            # ONLY weights so both vbs' head tiles land before the PE
            # finishes the DR runway
            for g in range(2, G):
                eng = nc.gpsimd if g % 2 == 0 else nc.scalar
                eng.dma_start(ht8_tiles[g][:], htt8[g, :, :, :])

            wh0 = wthp.tile([P, WHEAD, VBS], F16, name="wt_h", tag="wth")
            nc.sync.dma_start(wh0[:], wtb[0, :, 0:WHEAD, :])
            wh1 = wthp.tile([P, WHEAD, VBS], F16, name="wt_h", tag="wth")
            nc.sync.dma_start(wh1[:], wtb[1, :, 0:WHEAD, :])
            dma_ht_half(nc.gpsimd, 0, 0)
            dma_ht_half(nc.scalar, 1, 0)
            wb0 = wtbp.tile([P, KF - WHEAD, VBS], F16, name="wt_b", tag="wtb")
            nc.sync.dma_start(wb0[:], wtb[0, :, WHEAD:KF, :])
            dma_ht_half(nc.gpsimd, 0, 1)
            dma_ht_half(nc.scalar, 1, 1)
            wb1 = wtbp.tile([P, KF - WHEAD, VBS], F16, name="wt_b", tag="wtb")
            nc.sync.dma_start(wb1[:], wtb[1, :, WHEAD:KF, :])
            wt_head[0], wt_body[0] = wh0, wb0
            wt_head[1], wt_body[1] = wh1, wb1

            for dc in range(HSPLIT, KF):
                eng = nc.gpsimd if dc % 2 == 0 else nc.scalar
                eng.dma_start(ht_tiles[dc][:], htt[dc * P:(dc + 1) * P, :])

            def ht_slice(dc, tb):
                if dc < HSPLIT:
                    half = ht_halves[dc][tb // 2]
                    r = tb % 2
                    return half[:, r * TBS:(r + 1) * TBS]
                return ht_tiles[dc][:, tb * TBS:(tb + 1) * TBS]

            def wt_slice(vb, wt_t, dc):
                if vb in wt_head:
                    if dc < WHEAD:
                        return wt_head[vb][:, dc, :]
                    return wt_body[vb][:, dc - WHEAD, :]
                return wt_t[:, dc, :]

            def mm(vb, pss, wt_t, w8_t, dc, tb):
                """dc in [0, KF) -> fp16 chunk; dc in [KF, KF+G) -> fp8 pair."""
                ts0 = tb * TBS
                if dc < KF:
                    nc.tensor.matmul(
                        pss[tb][:],
                        wt_slice(vb, wt_t, dc),
                        ht_slice(dc, tb),
                        start=(G == 0 and dc == 0),
                        stop=(dc == KF - 1),
                    )
                else:
                    g = dc - KF
                    nc.tensor.matmul(
                        pss[tb][:],
                        w8_t[:, 2 * g:2 * g + 2, 0:VBS],
                        ht8_tiles[g][:, :, ts0:ts0 + TBS],
                        start=(g == 0),
                        stop=False,
                        perf_mode=DR,
                    )

            def evict_tb(vb, pss, tb):
                ts0 = tb * TBS
                ot = outp.tile([VBS, TBS], F16, name="ot", tag="ot")
                nc.vector.tensor_scalar_mul(ot[:], pss[tb][:], 1.0 / PSUM_SCALE)
                # alternate output DMAs over two queues for 2x drain rate
                eng = nc.scalar if tb % 2 == 0 else nc.gpsimd
                eng.dma_start(
                    outt[vb * VBS:(vb + 1) * VBS, ts0:ts0 + TBS], ot[:]
                )

            SEQ = list(range(KF, KF + G)) + list(range(KF))

            # vb 0+1 interleaved at dc granularity: PE does 8 matmuls per
            # arriving ht chunk, tracking the DMA stream without stalling.
            # vb1 is staggered 2 chunks behind vb0 so the early matmuls
            # only depend on wt0 while wt1 is still in flight.
            pss01 = [
                [psp.tile([VBS, TBS], F32, name=f"ps{v}_{tb}", tag="ps")
                 for tb in range(TB)]
                for v in range(2)
            ]
            for d in SEQ:
                for v in range(2):
                    for tb in range(TB):
                        mm(v, pss01[v], None, wt8_tiles[v], d, tb)
            for v in range(2):
                for tb in range(TB):
                    evict_tb(v, pss01[v], tb)

            for vb in range(2, VB):
                wt_t, w8_t = fetch_wt(vb)
                pss = [
                    psp.tile([VBS, TBS], F32, name=f"ps{tb}", tag="ps")
                    for tb in range(TB)
                ]
                if vb < VB - 1:
                    for dc in SEQ:
                        for tb in range(TB):
                            mm(vb, pss, wt_t, w8_t, dc, tb)
                    for tb in range(TB):
                        evict_tb(vb, pss, tb)
                else:
                    # last vb tb-major: each token block finishes early so
                    # its eviction + output DMA overlap the remaining MMs;
                    # the final block drains in quarters to shorten the tail
                    for tb in range(TB):
                        for dc in SEQ:
                            mm(vb, pss, wt_t, w8_t, dc, tb)
                        if tb < TB - 1:
                            evict_tb(vb, pss, tb)
                        else:
                            ts0 = tb * TBS
                            for q in range(4):
                                qs = q * (TBS // 4)
                                ot = outp.tile([VBS, TBS // 4], F16,
                                               name="otq", tag="ot")
                                nc.vector.tensor_scalar_mul(
                                    ot[:], pss[tb][:, qs:qs + TBS // 4],
                                    1.0 / PSUM_SCALE,
                                )
                                eng = nc.scalar if q % 2 == 0 else nc.gpsimd
                                eng.dma_start(
                                    outt[vb * VBS:(vb + 1) * VBS,
                                         ts0 + qs:ts0 + qs + TBS // 4],
                                    ot[:],
                                )
    nc.compile()
    return nc


def _q8(x, scale):
    return np.clip(x * scale, -240.0, 240.0).astype(E4M3)


def _prep_inputs(hidden_states, weight, lora_A, lora_B):
    wf = np.asarray(weight, dtype=np.float32) + (
        np.asarray(lora_B, dtype=np.float32) @ np.asarray(lora_A, dtype=np.float32)
    )
    h32 = np.asarray(hidden_states, dtype=np.float32)

    # fp16 part: chunks [0, KF), weights pre-scaled by PSUM_SCALE
    ws = (wf[:, :KF * P] * np.float32(PSUM_SCALE)).astype(np.float16)
    # [core, vb, j, dc, p] -> [core, vb, p, dc, j]
    wtb_all = np.ascontiguousarray(
        ws.reshape(NCORES, VB, VBS, KF, P).transpose(0, 1, 4, 3, 2)
    )
    htt = np.ascontiguousarray(h32[:, :KF * P].astype(np.float16).T)

    maps = [{"wtb": wtb_all[c], "htt": htt} for c in range(NCORES)]

    if G:
        # fp8 part: chunks [KF, DC) as G DoubleRow pairs
        w8 = _q8(wf[:, KF * P:], W8_SCALE)          # [V, K8*P]
        # [c, vb, j, g, ko, p] -> [c, vb, p, (g,ko), j] padded j: 125->128
        w8b = w8.reshape(NCORES, VB, VBS, G, 2, P).transpose(0, 1, 5, 3, 4, 2)
        w8b = np.ascontiguousarray(
            np.pad(w8b, [(0, 0)] * 5 + [(0, P - VBS)])
            .reshape(NCORES, VB, P, 2 * G, P)
        )
        h8 = _q8(h32[:, KF * P:], H8_SCALE)         # [N, K8*P]
        # -> [g, ko, p, t] -> [g, p, ko, t]
        htt8 = np.ascontiguousarray(
            h8.T.reshape(G, 2, P, N_TOK).transpose(0, 2, 1, 3)
        )
        for c in range(NCORES):
            maps[c]["wtb8"] = w8b[c]
            maps[c]["htt8"] = htt8

    return maps


def run(hidden_states, weight, lora_A, lora_B, trace=False, **run_kwargs):
    in_maps = _prep_inputs(hidden_states, weight, lora_A, lora_B)
    nc = build_nc()
    res = run_bass_kernel_spmd(
        nc, in_maps, core_ids=list(range(NCORES)), trace=trace, **run_kwargs
    )
    out = np.empty((N_TOK, V), dtype=np.float32)
    for c in range(NCORES):
        out[:, c * VC:(c + 1) * VC] = res.results[c]["outt"].T.astype(np.float32)
    return out, res


def kernel(hidden_states, weight, lora_A, lora_B):
    out, _ = run(hidden_states, weight, lora_A, lora_B, trace=False)
    return out
